# revision 13
# baseline (speedup 1.0000x reference)
"""Trainium2 Bass kernel for nn_Attention_GLM_Wrapped (S=2048, B=2, D=4096, H=32).

Sharding: 8-way tensor parallel over heads (4 heads/core), both batches on
every core. Per-batch AllToAll redistributes the attention output from
head-sharded to token-sharded form for the output projection; each core
emits the final output rows for its 256-token slice (both batches).

Per-core pipeline (SPMD, identical program, per-core weight shards):
  A) Fused Q/K/V projection in natural [token, e] layout (all three weight
     blocks resident in SBUF, x streamed once in 128-token blocks), bias,
     2D-RoPE on Q/K via free-dim shifted vector ops, PE-transpose of Q/K to
     [d, token] layout, spill to DRAM.
  C) Per (batch, head): logits^T = K^T-chunk @ Q (PSUM), exp on ACT over
     1024-query tiles (no max subtraction; logits are O(10) so exp is safe),
     all-ones-matmul key-sum (replicated over partitions), P@V accumulation,
     normalize with fast reciprocal.  AllToAll for batch b is issued as soon
     as batch b's heads finish, overlapping batch b+1's attention.
  D) Output projection per batch against full attn_out_weight^T; bias.

Matmuls run in float16 (fp32 PSUM accumulation); 4-byte operands stream at
half PE rate on TRN2, so 2-byte operands double matmul throughput vs
fp32/fp32r, and fp16 carries 2 more mantissa bits than bf16. A constant
-10 offset on the logits keeps exp outputs inside fp16 range (the offset
cancels exactly in the softmax normalization).
"""
import os
import sys

sys.path.insert(0, "/opt/trn_rl_repo")

import numpy as np
import ml_dtypes
from contextlib import ExitStack

import concourse.bass as bass
from concourse import bacc
import concourse.mybir as mybir
import concourse.tile as tile
from concourse.bass_utils import run_bass_kernel_spmd
from concourse.masks import make_identity

F32 = mybir.dt.float32
F32R = mybir.dt.float32r
BF16 = mybir.dt.bfloat16
FP16 = mybir.dt.float16
AF = mybir.ActivationFunctionType

MMD = FP16          # matmul operand dtype
EXPB = -10.0        # constant logit offset before exp; cancels in softmax

S, B, D = 2048, 2, 4096
H = 32
HD = 128            # head dim
T = S * B           # 4096 tokens, t = b*S + s
NC = 8              # cores
HPC = H // NC       # 4 heads per core
EH = HPC * HD       # 512 local e-dims per q/k/v
TPC = T // NC // B  # 256 tokens per core per batch (output shard)
SCALE = float(1.0 / np.sqrt(HD))

_cache = {}


def _np_mmd(a):
    if MMD == BF16:
        return np.asarray(a, np.float32).astype(ml_dtypes.bfloat16)
    if MMD == FP16:
        return np.asarray(a, np.float32).astype(np.float16)
    return np.ascontiguousarray(np.asarray(a, np.float32))


def _rope_tables():
    rot = 64
    inv_freq = 1.0 / (10000.0 ** (np.arange(0, rot, 2, dtype=np.float32) / rot))
    v = np.arange(S, dtype=np.float32)[:, None] * inv_freq[None, :]
    v = np.concatenate([v, v], axis=-1)  # [S, 64]
    return np.cos(v).astype(np.float32), np.sin(v).astype(np.float32)


def build_program():
    nc = bacc.Bacc("TRN2", target_bir_lowering=False, debug=False, num_devices=NC)

    xT = nc.dram_tensor("xT", [D, T], MMD, kind="ExternalInput").ap()
    wqT = nc.dram_tensor("wqT", [D, EH], MMD, kind="ExternalInput").ap()
    wkT = nc.dram_tensor("wkT", [D, EH], MMD, kind="ExternalInput").ap()
    wvT = nc.dram_tensor("wvT", [D, EH], MMD, kind="ExternalInput").ap()
    woT = nc.dram_tensor("woT", [D, D], MMD, kind="ExternalInput").ap()
    bq = nc.dram_tensor("bq", [HD, EH], F32, kind="ExternalInput").ap()
    bk = nc.dram_tensor("bk", [HD, EH], F32, kind="ExternalInput").ap()
    bv = nc.dram_tensor("bv", [HD, EH], F32, kind="ExternalInput").ap()
    bo = nc.dram_tensor("bo", [HD, D], F32, kind="ExternalInput").ap()
    cosN = nc.dram_tensor("cosN", [T, HD], F32, kind="ExternalInput").ap()
    sinN = nc.dram_tensor("sinN", [T, HD], F32, kind="ExternalInput").ap()
    onesc = nc.dram_tensor("onesc", [HD, HD], MMD, kind="ExternalInput").ap()
    out = nc.dram_tensor("out", [B, TPC, D], F32, kind="ExternalOutput").ap()
    DEBUG = bool(int(os.environ.get("K_DEBUG", "0")))
    if DEBUG:
        qdump = nc.dram_tensor("qdump", [EH, T], MMD, kind="ExternalOutput").ap()
        kdump = nc.dram_tensor("kdump", [EH, T], MMD, kind="ExternalOutput").ap()
        vdump = nc.dram_tensor("vdump", [T, EH], MMD, kind="ExternalOutput").ap()
        ccdump = nc.dram_tensor("ccdump", [B, NC, EH, TPC], MMD, kind="ExternalOutput").ap()

    NTB = T // HD   # 32 token blocks of 128
    NDC = D // HD   # 32 contraction chunks

    with tile.TileContext(nc) as tc, ExitStack() as top:
        dram = top.enter_context(tc.tile_pool(name="dram", bufs=1, space="DRAM"))
        cpool = top.enter_context(tc.tile_pool(name="cpool", bufs=1))

        qT_d = dram.tile([EH, T], MMD)
        kT_d = dram.tile([EH, T], MMD)
        v_d = dram.tile([T, EH], MMD)
        cc_in = [dram.tile([NC, EH, TPC], MMD, name=f"cc_in_{b}")
                 for b in range(B)]
        cc_out = [dram.tile([NC, EH, TPC], MMD, name=f"cc_out_{b}")
                  for b in range(B)]

        ident = cpool.tile([HD, HD], MMD)
        make_identity(nc, ident)
        ones_sb = cpool.tile([HD, HD], MMD)
        nc.sync.dma_start(ones_sb[:], onesc[:])
        bq_sb = cpool.tile([HD, EH], F32)
        nc.sync.dma_start(bq_sb[:], bq[:])
        bk_sb = cpool.tile([HD, EH], F32)
        nc.sync.dma_start(bk_sb[:], bk[:])
        bv_sb = cpool.tile([HD, EH], F32)
        nc.sync.dma_start(bv_sb[:], bv[:])
        expb_sb = cpool.tile([HD, 1], F32)
        nc.vector.memset(expb_sb[:], EXPB)

        xT_r = xT.rearrange("(o p) t -> p o t", p=HD)        # [128, 32, T]

        # ------- Phase A: fused Q/K/V projection + RoPE + transpose --------
        with ExitStack() as ctx:
            wres = ctx.enter_context(tc.tile_pool(name="wres", bufs=1))
            xp = ctx.enter_context(tc.tile_pool(name="xp", bufs=3))
            rp = ctx.enter_context(tc.tile_pool(name="rp", bufs=3))
            op = ctx.enter_context(tc.tile_pool(name="op", bufs=6))
            ps = ctx.enter_context(tc.tile_pool(name="psA", bufs=4, space="PSUM"))
            pst = ctx.enter_context(tc.tile_pool(name="psAt", bufs=4, space="PSUM"))

            # chunked preloads so the first matmuls only wait for chunk 0
            wqS = wres.tile([HD, NDC, EH], MMD)
            wkS = wres.tile([HD, NDC, EH], MMD)
            wvS = wres.tile([HD, NDC, EH], MMD)
            for ch in range(8):
                csl = slice(ch * NDC // 8, (ch + 1) * NDC // 8)
                for wS, wsrc in ((wqS, wqT), (wkS, wkT), (wvS, wvT)):
                    r = wsrc.rearrange("(o p) e -> p o e", p=HD)
                    nc.sync.dma_start(wS[:, csl], r[:, csl])

            for tb in range(NTB):
                tsl = slice(tb * HD, (tb + 1) * HD)
                xo = xp.tile([HD, NDC, HD], MMD, tag="xo")
                nc.scalar.dma_start(xo[:], xT_r[:, :, tsl])
                cos_t = xp.tile([HD, HD], F32, tag="cos")
                nc.scalar.dma_start(cos_t[:], cosN[tsl, :])
                sin_t = xp.tile([HD, HD], F32, tag="sin")
                nc.scalar.dma_start(sin_t[:], sinN[tsl, :])

                for name, wS, b_sb in (("q", wqS, bq_sb), ("k", wkS, bk_sb)):
                    outd = qT_d if name == "q" else kT_d
                    pq = ps.tile([HD, EH], F32, tag="pqk", name=f"pqk_{name}_{tb}")
                    for d in range(NDC):
                        nc.tensor.matmul(pq[:], xo[:, d], wS[:, d],
                                         start=(d == 0), stop=(d == NDC - 1))
                    qb = rp.tile([HD, EH], F32, tag="qb", name=f"qb_{name}_{tb}")
                    nc.vector.tensor_tensor(
                        qb[:], pq[:], b_sb[:], mybir.AluOpType.add)
                    # rope: rq = qb*cos + shift(qb)*sin_signed
                    rq = rp.tile([HD, EH], MMD, tag="rq", name=f"rq_{name}_{tb}")
                    qb4 = qb.rearrange("p (h e) -> p h e", h=HPC)
                    rq4 = rq.rearrange("p (h e) -> p h e", h=HPC)
                    cosb = cos_t[:, None, :].to_broadcast([HD, HPC, HD])
                    nc.vector.tensor_tensor(rq4[:], qb4[:], cosb,
                                            mybir.AluOpType.mult)
                    qb8 = qb.rearrange("p (h u e) -> p h u e", h=HPC, u=4)
                    rq8 = rq.rearrange("p (h u e) -> p h u e", h=HPC, u=4)
                    sin8 = sin_t.rearrange("p (u e) -> p u e", u=4)
                    tmp = rp.tile([HD, HPC, 2, 32], F32, tag="tmp",
                                  name=f"tmp_{name}_{tb}")
                    nc.vector.tensor_tensor(
                        tmp[:], qb8[:, :, 1::2, :],
                        sin8[:, None, 0::2, :].to_broadcast([HD, HPC, 2, 32]),
                        mybir.AluOpType.mult)
                    nc.vector.tensor_tensor(
                        rq8[:, :, 0::2, :], rq8[:, :, 0::2, :], tmp[:],
                        mybir.AluOpType.add)
                    nc.vector.tensor_tensor(
                        tmp[:], qb8[:, :, 0::2, :],
                        sin8[:, None, 1::2, :].to_broadcast([HD, HPC, 2, 32]),
                        mybir.AluOpType.mult)
                    nc.vector.tensor_tensor(
                        rq8[:, :, 1::2, :], rq8[:, :, 1::2, :], tmp[:],
                        mybir.AluOpType.add)
                    # transpose each head block to [d, tok] and spill
                    for hl in range(HPC):
                        ptr = pst.tile([HD, HD], MMD, tag="ptr",
                                       name=f"ptr_{name}_{tb}_{hl}")
                        nc.tensor.transpose(ptr[:], rq[:, hl * HD:(hl + 1) * HD],
                                            ident[:])
                        ob = op.tile([HD, HD], MMD, tag="ob",
                                     name=f"ob_{name}_{tb}_{hl}")
                        nc.scalar.copy(ob[:], ptr[:])
                        nc.sync.dma_start(outd[hl * HD:(hl + 1) * HD, tsl], ob[:])

                # V: natural layout, bias only
                pv = ps.tile([HD, EH], F32, tag="pqk", name=f"pv_{tb}")
                for d in range(NDC):
                    nc.tensor.matmul(pv[:], xo[:, d], wvS[:, d],
                                     start=(d == 0), stop=(d == NDC - 1))
                vb = op.tile([HD, EH], MMD, tag="vb", name=f"vb_{tb}")
                nc.vector.tensor_tensor(
                    vb[:], pv[:], bv_sb[:], mybir.AluOpType.add)
                nc.sync.dma_start(v_d[tsl, :], vb[:])

        # ------- Phase C: attention per (batch, head) + per-batch A2A ------
        # Phase D pools are opened alongside C so D's input DMAs can be
        # emitted (on the otherwise-idle GPSIMD queue) right after each
        # AllToAll -- every other engine queue is still clogged with phase C
        # work at that point, and HWDGE issuance is in-order per engine.
        NKC = S // HD    # 16 key chunks
        NQT = S // 512   # 4 query tiles of 512
        NES = D // 512   # 8 output column segments
        with ExitStack() as ctx:
            qk = ctx.enter_context(tc.tile_pool(name="qk", bufs=2))
            pp = ctx.enter_context(tc.tile_pool(name="pp", bufs=6))
            ao = ctx.enter_context(tc.tile_pool(name="ao", bufs=4))
            wvf = ctx.enter_context(tc.tile_pool(name="wvf", bufs=1))
            wop = ctx.enter_context(tc.tile_pool(name="wop", bufs=4))
            oo = ctx.enter_context(tc.tile_pool(name="oo", bufs=4))

            bo_sb = wvf.tile([HD, D], F32)
            nc.sync.dma_start(bo_sb[:], bo[:])
            woT_r = woT.rearrange("(o p) e -> p o e", p=HD)  # [128, 32, D]
            wvfS = {}
            wo_first = {}

            with ExitStack() as cps:
                psl = cps.enter_context(tc.tile_pool(name="psl", bufs=4, space="PSUM"))
                pso = cps.enter_context(tc.tile_pool(name="pso", bufs=2, space="PSUM"))
                pss = cps.enter_context(tc.tile_pool(name="pss", bufs=2, space="PSUM"))

                for b in range(B):
                    ssl = slice(b * S, (b + 1) * S)
                    for hl in range(HPC):
                        esl = slice(hl * HD, (hl + 1) * HD)
                        qh = qk.tile([HD, S], MMD, tag="qh", name=f"qh_{b}_{hl}")
                        nc.sync.dma_start(qh[:], qT_d[esl, ssl])
                        kh = qk.tile([HD, S], MMD, tag="kh", name=f"kh_{b}_{hl}")
                        nc.sync.dma_start(kh[:], kT_d[esl, ssl])
                        vh = qk.tile([HD, NKC, HD], MMD, tag="vh", name=f"vh_{b}_{hl}")
                        nc.sync.dma_start(
                            vh[:], v_d[ssl, esl].rearrange("(o p) e -> p o e", p=HD))

                        for qt in range(NQT):
                            qsl = slice(qt * 512, (qt + 1) * 512)
                            po = pso.tile([HD, 512], F32, tag="po",
                                          name=f"po_{b}_{hl}_{qt}")
                            su = pss.tile([HD, 512], F32, tag="su",
                                          name=f"su_{b}_{hl}_{qt}")
                            for kc in range(NKC):
                                pl = psl.tile([HD, 512], F32, tag="pl",
                                              name=f"pl_{b}_{hl}_{qt}_{kc}")
                                nc.tensor.matmul(
                                    pl[:], kh[:, kc * HD:(kc + 1) * HD], qh[:, qsl],
                                    start=True, stop=True)
                                pe = pp.tile([HD, 512], MMD, tag="pe",
                                             name=f"pe_{b}_{hl}_{qt}_{kc}")
                                nc.scalar.activation(pe[:], pl[:], AF.Exp,
                                                     scale=SCALE, bias=expb_sb[:])
                                nc.tensor.matmul(su[:], ones_sb[:], pe[:],
                                                 start=(kc == 0),
                                                 stop=(kc == NKC - 1))
                                nc.tensor.matmul(po[:], vh[:, kc], pe[:],
                                                 start=(kc == 0),
                                                 stop=(kc == NKC - 1))
                            rec = ao.tile([HD, 512], F32, tag="rec",
                                          name=f"rec_{b}_{hl}_{qt}")
                            nc.vector.reciprocal_approx_fast(rec[:], su[:])
                            osb = ao.tile([HD, 512], MMD, tag="osb",
                                          name=f"osb_{b}_{hl}_{qt}")
                            nc.vector.tensor_tensor(
                                osb[:], po[:], rec[:], mybir.AluOpType.mult)
                            for j2 in range(2):
                                j = qt * 2 + j2
                                nc.sync.dma_start(
                                    cc_in[b][j, esl, :],
                                    osb[:, j2 * TPC:(j2 + 1) * TPC])
                    # batch b attention done: exchange while b+1 computes
                    nc.gpsimd.collective_compute(
                        "AllToAll", mybir.AluOpType.bypass,
                        replica_groups=[list(range(NC))],
                        ins=[cc_in[b][:]], outs=[cc_out[b][:]],
                    )
                    # prefetch phase-D inputs for this batch on the Pool queue
                    t_ = wvf.tile([HD, NDC, TPC], MMD, name=f"wvfS_{b}")
                    for i in range(NC):
                        nc.gpsimd.dma_start(
                            t_[:, i * HPC:(i + 1) * HPC, :],
                            cc_out[b][i].rearrange("(r1 p) c -> p r1 c", p=HD))
                    wvfS[b] = t_
                    wlo = wop.tile([HD, NDC // 2, 512], MMD, tag="wo",
                                   name=f"wo_pre_lo_{b}")
                    nc.gpsimd.dma_start(wlo[:], woT_r[:, 0:NDC // 2, 0:512])
                    whi = wop.tile([HD, NDC // 2, 512], MMD, tag="wo",
                                   name=f"wo_pre_hi_{b}")
                    nc.gpsimd.dma_start(whi[:], woT_r[:, NDC // 2:NDC, 0:512])
                    wo_first[b] = (wlo, whi)

            if DEBUG:
                nc.sync.dma_start(qdump[:], qT_d[:])
                nc.sync.dma_start(kdump[:], kT_d[:])
                nc.sync.dma_start(vdump[:], v_d[:])
                for b in range(B):
                    nc.sync.dma_start(ccdump[b], cc_out[b][:])

            # ------- Phase D: output projection per batch ------------------
            with ExitStack() as dps:
                ps = dps.enter_context(tc.tile_pool(name="psD", bufs=4, space="PSUM"))
                for b in range(B):
                    for es in range(NES):
                        esl = slice(es * 512, (es + 1) * 512)
                        if es == 0:
                            wo_lo, wo_hi = wo_first[b]
                        else:
                            wo_lo = wop.tile([HD, NDC // 2, 512], MMD, tag="wo")
                            nc.sync.dma_start(wo_lo[:], woT_r[:, 0:NDC // 2, esl])
                            wo_hi = wop.tile([HD, NDC // 2, 512], MMD, tag="wo")
                            nc.sync.dma_start(wo_hi[:], woT_r[:, NDC // 2:NDC, esl])
                        for tb2 in range(TPC // HD):
                            pd = ps.tile([HD, 512], F32, tag="pd",
                                         name=f"pd_{b}_{es}_{tb2}")
                            for d in range(NDC):
                                wo_t = wo_lo if d < NDC // 2 else wo_hi
                                nc.tensor.matmul(
                                    pd[:],
                                    wvfS[b][:, d, tb2 * HD:(tb2 + 1) * HD],
                                    wo_t[:, d % (NDC // 2)],
                                    start=(d == 0), stop=(d == NDC - 1))
                            ob = oo.tile([HD, 512], F32, tag="obD",
                                         name=f"obD_{b}_{es}_{tb2}")
                            nc.vector.tensor_tensor(
                                ob[:], pd[:], bo_sb[:, esl], mybir.AluOpType.add)
                            nc.sync.dma_start(
                                out[b, tb2 * HD:(tb2 + 1) * HD, esl], ob[:])

    nc.compile()
    return nc


def host_prep(x, position_ids, qkv_weight, qkv_bias, attn_out_weight,
              attn_out_bias):
    pos = np.asarray(position_ids).astype(np.int64)
    x = np.asarray(x, dtype=np.float32)
    Wqkv = np.asarray(qkv_weight, dtype=np.float32)
    bqkv = np.asarray(qkv_bias, dtype=np.float32)
    Wo = np.asarray(attn_out_weight, dtype=np.float32)
    bo = np.asarray(attn_out_bias, dtype=np.float32)

    xT = _np_mmd(x.transpose(2, 1, 0).reshape(D, T))
    woT = _np_mmd(Wo.T)

    cos_t, sin_t = _rope_tables()
    cosN = np.empty((T, HD), np.float32)
    sinN = np.empty((T, HD), np.float32)
    for b in range(B):
        rows = slice(b * S, (b + 1) * S)
        p1 = pos[b, 0, :]
        p2 = pos[b, 1, :]
        cosN[rows, 0:64] = cos_t[p1]
        cosN[rows, 64:128] = cos_t[p2]
        s1 = sin_t[p1].copy()
        s1[:, 0:32] *= -1.0
        s2 = sin_t[p2].copy()
        s2[:, 0:32] *= -1.0
        sinN[rows, 0:64] = s1
        sinN[rows, 64:128] = s2

    ones = _np_mmd(np.ones((HD, HD), np.float32))
    shared = dict(xT=xT, woT=woT, cosN=cosN, sinN=sinN, onesc=ones,
                  bo=np.ascontiguousarray(np.broadcast_to(bo, (HD, D))))

    in_maps = []
    for c in range(NC):
        heads = range(HPC * c, HPC * (c + 1))
        wq = np.concatenate([Wqkv[384 * h: 384 * h + 128] for h in heads])
        wk = np.concatenate([Wqkv[384 * h + 128: 384 * h + 256] for h in heads])
        wv = np.concatenate([Wqkv[384 * h + 256: 384 * h + 384] for h in heads])
        in_maps.append(dict(
            shared,
            wqT=_np_mmd(wq.T), wkT=_np_mmd(wk.T), wvT=_np_mmd(wv.T),
            bq=np.ascontiguousarray(np.broadcast_to(np.concatenate(
                [bqkv[384 * h: 384 * h + 128] for h in heads]), (HD, EH))),
            bk=np.ascontiguousarray(np.broadcast_to(np.concatenate(
                [bqkv[384 * h + 128: 384 * h + 256] for h in heads]), (HD, EH))),
            bv=np.ascontiguousarray(np.broadcast_to(np.concatenate(
                [bqkv[384 * h + 256: 384 * h + 384] for h in heads]), (HD, EH))),
        ))
    return in_maps


def kernel(x, position_ids, qkv_weight, qkv_bias, attn_out_weight,
           attn_out_bias, _trace=False):
    if "nc" not in _cache:
        _cache["nc"] = build_program()
    nc = _cache["nc"]

    in_maps = host_prep(x, position_ids, qkv_weight, qkv_bias,
                        attn_out_weight, attn_out_bias)
    res = run_bass_kernel_spmd(nc, in_maps, core_ids=list(range(NC)),
                               trace=_trace)
    _cache["last_result"] = res

    out = np.empty((S, B, D), np.float32)
    for c in range(NC):
        oc = res.results[c]["out"]  # [B, TPC, D]
        for b in range(B):
            out[TPC * c: TPC * (c + 1), b, :] = oc[b]
    return out


# revision 14
# speedup vs baseline: 1.0131x; 1.0131x over previous
"""Trainium2 Bass kernel for nn_Attention_GLM_Wrapped (S=2048, B=2, D=4096, H=32).

Sharding: 8-way tensor parallel over heads (4 heads/core), both batches on
every core. Per-batch AllToAll redistributes the attention output from
head-sharded to token-sharded form for the output projection; each core
emits the final output rows for its 256-token slice (both batches).

Per-core pipeline (SPMD, identical program, per-core weight shards):
  A) Fused Q/K/V projection in natural [token, e] layout (all three weight
     blocks resident in SBUF, x streamed once in 128-token blocks), bias,
     2D-RoPE on Q/K via free-dim shifted vector ops, PE-transpose of Q/K to
     [d, token] layout, spill to DRAM.
  C) Per (batch, head): logits^T = K^T-chunk @ Q (PSUM), exp on ACT over
     1024-query tiles (no max subtraction; logits are O(10) so exp is safe),
     all-ones-matmul key-sum (replicated over partitions), P@V accumulation,
     normalize with fast reciprocal.  AllToAll for batch b is issued as soon
     as batch b's heads finish, overlapping batch b+1's attention.
  D) Output projection per batch against full attn_out_weight^T; bias.

Matmuls run in float16 (fp32 PSUM accumulation); 4-byte operands stream at
half PE rate on TRN2, so 2-byte operands double matmul throughput vs
fp32/fp32r, and fp16 carries 2 more mantissa bits than bf16. A constant
-10 offset on the logits keeps exp outputs inside fp16 range (the offset
cancels exactly in the softmax normalization).
"""
import os
import sys

sys.path.insert(0, "/opt/trn_rl_repo")

import numpy as np
import ml_dtypes
from contextlib import ExitStack

import concourse.bass as bass
from concourse import bacc
import concourse.mybir as mybir
import concourse.tile as tile
from concourse.bass_utils import run_bass_kernel_spmd
from concourse.masks import make_identity

F32 = mybir.dt.float32
F32R = mybir.dt.float32r
BF16 = mybir.dt.bfloat16
FP16 = mybir.dt.float16
AF = mybir.ActivationFunctionType

MMD = FP16          # matmul operand dtype
EXPB = -10.0        # constant logit offset before exp; cancels in softmax

S, B, D = 2048, 2, 4096
H = 32
HD = 128            # head dim
T = S * B           # 4096 tokens, t = b*S + s
NC = 8              # cores
HPC = H // NC       # 4 heads per core
EH = HPC * HD       # 512 local e-dims per q/k/v
TPC = T // NC // B  # 256 tokens per core per batch (output shard)
SCALE = float(1.0 / np.sqrt(HD))

_cache = {}


def _np_mmd(a):
    if MMD == BF16:
        return np.asarray(a, np.float32).astype(ml_dtypes.bfloat16)
    if MMD == FP16:
        return np.asarray(a, np.float32).astype(np.float16)
    return np.ascontiguousarray(np.asarray(a, np.float32))


def _rope_tables():
    rot = 64
    inv_freq = 1.0 / (10000.0 ** (np.arange(0, rot, 2, dtype=np.float32) / rot))
    v = np.arange(S, dtype=np.float32)[:, None] * inv_freq[None, :]
    v = np.concatenate([v, v], axis=-1)  # [S, 64]
    return np.cos(v).astype(np.float32), np.sin(v).astype(np.float32)


def build_program():
    nc = bacc.Bacc("TRN2", target_bir_lowering=False, debug=False, num_devices=NC)

    xT = nc.dram_tensor("xT", [D, T], MMD, kind="ExternalInput").ap()
    wqT = nc.dram_tensor("wqT", [D, EH], MMD, kind="ExternalInput").ap()
    wkT = nc.dram_tensor("wkT", [D, EH], MMD, kind="ExternalInput").ap()
    wvT = nc.dram_tensor("wvT", [D, EH], MMD, kind="ExternalInput").ap()
    woT = nc.dram_tensor("woT", [D, D], MMD, kind="ExternalInput").ap()
    bq = nc.dram_tensor("bq", [HD, EH], F32, kind="ExternalInput").ap()
    bk = nc.dram_tensor("bk", [HD, EH], F32, kind="ExternalInput").ap()
    bv = nc.dram_tensor("bv", [HD, EH], F32, kind="ExternalInput").ap()
    bo = nc.dram_tensor("bo", [HD, D], F32, kind="ExternalInput").ap()
    cosN = nc.dram_tensor("cosN", [T, HD], F32, kind="ExternalInput").ap()
    sinN = nc.dram_tensor("sinN", [T, HD], F32, kind="ExternalInput").ap()
    onesc = nc.dram_tensor("onesc", [HD, HD], MMD, kind="ExternalInput").ap()
    out = nc.dram_tensor("out", [B, TPC, D], F32, kind="ExternalOutput").ap()
    DEBUG = bool(int(os.environ.get("K_DEBUG", "0")))
    if DEBUG:
        qdump = nc.dram_tensor("qdump", [EH, T], MMD, kind="ExternalOutput").ap()
        kdump = nc.dram_tensor("kdump", [EH, T], MMD, kind="ExternalOutput").ap()
        vdump = nc.dram_tensor("vdump", [T, EH], MMD, kind="ExternalOutput").ap()
        ccdump = nc.dram_tensor("ccdump", [B, NC, EH, TPC], MMD, kind="ExternalOutput").ap()

    NTB = T // HD   # 32 token blocks of 128
    NDC = D // HD   # 32 contraction chunks

    with tile.TileContext(nc) as tc, ExitStack() as top:
        dram = top.enter_context(tc.tile_pool(name="dram", bufs=1, space="DRAM"))
        cpool = top.enter_context(tc.tile_pool(name="cpool", bufs=1))

        qT_d = dram.tile([EH, T], MMD)
        kT_d = dram.tile([EH, T], MMD)
        v_d = dram.tile([T, EH], MMD)
        cc_in = [dram.tile([NC, EH, TPC], MMD, name=f"cc_in_{b}")
                 for b in range(B)]
        cc_out = [dram.tile([NC, EH, TPC], MMD, name=f"cc_out_{b}")
                  for b in range(B)]

        ident = cpool.tile([HD, HD], MMD)
        make_identity(nc, ident)
        ones_sb = cpool.tile([HD, HD], MMD)
        nc.sync.dma_start(ones_sb[:], onesc[:])
        bq_sb = cpool.tile([HD, EH], F32)
        nc.sync.dma_start(bq_sb[:], bq[:])
        bk_sb = cpool.tile([HD, EH], F32)
        nc.sync.dma_start(bk_sb[:], bk[:])
        bv_sb = cpool.tile([HD, EH], F32)
        nc.sync.dma_start(bv_sb[:], bv[:])
        expb_sb = cpool.tile([HD, 1], F32)
        nc.vector.memset(expb_sb[:], EXPB)

        xT_r = xT.rearrange("(o p) t -> p o t", p=HD)        # [128, 32, T]

        # ------- Phase A: fused Q/K/V projection + RoPE + transpose --------
        with ExitStack() as ctx:
            wres = ctx.enter_context(tc.tile_pool(name="wres", bufs=1))
            xp = ctx.enter_context(tc.tile_pool(name="xp", bufs=3))
            rp = ctx.enter_context(tc.tile_pool(name="rp", bufs=3))
            op = ctx.enter_context(tc.tile_pool(name="op", bufs=6))
            ps = ctx.enter_context(tc.tile_pool(name="psA", bufs=4, space="PSUM"))
            pst = ctx.enter_context(tc.tile_pool(name="psAt", bufs=4, space="PSUM"))

            # chunked preloads so the first matmuls only wait for chunk 0
            wqS = wres.tile([HD, NDC, EH], MMD)
            wkS = wres.tile([HD, NDC, EH], MMD)
            wvS = wres.tile([HD, NDC, EH], MMD)
            for ch in range(8):
                csl = slice(ch * NDC // 8, (ch + 1) * NDC // 8)
                for wS, wsrc in ((wqS, wqT), (wkS, wkT), (wvS, wvT)):
                    r = wsrc.rearrange("(o p) e -> p o e", p=HD)
                    nc.sync.dma_start(wS[:, csl], r[:, csl])

            for tb in range(NTB):
                tsl = slice(tb * HD, (tb + 1) * HD)
                xo = xp.tile([HD, NDC, HD], MMD, tag="xo")
                nc.scalar.dma_start(xo[:], xT_r[:, :, tsl])
                cos_t = xp.tile([HD, HD], F32, tag="cos")
                nc.scalar.dma_start(cos_t[:], cosN[tsl, :])
                sin_t = xp.tile([HD, HD], F32, tag="sin")
                nc.scalar.dma_start(sin_t[:], sinN[tsl, :])

                for name, wS, b_sb in (("q", wqS, bq_sb), ("k", wkS, bk_sb)):
                    outd = qT_d if name == "q" else kT_d
                    pq = ps.tile([HD, EH], F32, tag="pqk", name=f"pqk_{name}_{tb}")
                    for d in range(NDC):
                        nc.tensor.matmul(pq[:], xo[:, d], wS[:, d],
                                         start=(d == 0), stop=(d == NDC - 1))
                    qb = rp.tile([HD, EH], F32, tag="qb", name=f"qb_{name}_{tb}")
                    nc.vector.tensor_tensor(
                        qb[:], pq[:], b_sb[:], mybir.AluOpType.add)
                    # rope: rq = qb*cos + shift(qb)*sin_signed
                    rq = rp.tile([HD, EH], MMD, tag="rq", name=f"rq_{name}_{tb}")
                    qb4 = qb.rearrange("p (h e) -> p h e", h=HPC)
                    rq4 = rq.rearrange("p (h e) -> p h e", h=HPC)
                    cosb = cos_t[:, None, :].to_broadcast([HD, HPC, HD])
                    nc.vector.tensor_tensor(rq4[:], qb4[:], cosb,
                                            mybir.AluOpType.mult)
                    qb8 = qb.rearrange("p (h u e) -> p h u e", h=HPC, u=4)
                    rq8 = rq.rearrange("p (h u e) -> p h u e", h=HPC, u=4)
                    sin8 = sin_t.rearrange("p (u e) -> p u e", u=4)
                    tmp = rp.tile([HD, HPC, 2, 32], F32, tag="tmp",
                                  name=f"tmp_{name}_{tb}")
                    nc.vector.tensor_tensor(
                        tmp[:], qb8[:, :, 1::2, :],
                        sin8[:, None, 0::2, :].to_broadcast([HD, HPC, 2, 32]),
                        mybir.AluOpType.mult)
                    nc.vector.tensor_tensor(
                        rq8[:, :, 0::2, :], rq8[:, :, 0::2, :], tmp[:],
                        mybir.AluOpType.add)
                    nc.vector.tensor_tensor(
                        tmp[:], qb8[:, :, 0::2, :],
                        sin8[:, None, 1::2, :].to_broadcast([HD, HPC, 2, 32]),
                        mybir.AluOpType.mult)
                    nc.vector.tensor_tensor(
                        rq8[:, :, 1::2, :], rq8[:, :, 1::2, :], tmp[:],
                        mybir.AluOpType.add)
                    # transpose each head block to [d, tok] and spill
                    for hl in range(HPC):
                        ptr = pst.tile([HD, HD], MMD, tag="ptr",
                                       name=f"ptr_{name}_{tb}_{hl}")
                        nc.tensor.transpose(ptr[:], rq[:, hl * HD:(hl + 1) * HD],
                                            ident[:])
                        ob = op.tile([HD, HD], MMD, tag="ob",
                                     name=f"ob_{name}_{tb}_{hl}")
                        nc.scalar.copy(ob[:], ptr[:])
                        nc.sync.dma_start(outd[hl * HD:(hl + 1) * HD, tsl], ob[:])

                # V: natural layout, bias only
                pv = ps.tile([HD, EH], F32, tag="pqk", name=f"pv_{tb}")
                for d in range(NDC):
                    nc.tensor.matmul(pv[:], xo[:, d], wvS[:, d],
                                     start=(d == 0), stop=(d == NDC - 1))
                vb = op.tile([HD, EH], MMD, tag="vb", name=f"vb_{tb}")
                nc.vector.tensor_tensor(
                    vb[:], pv[:], bv_sb[:], mybir.AluOpType.add)
                nc.sync.dma_start(v_d[tsl, :], vb[:])

        # ------- Phase C: attention per (batch, head) + per-batch A2A ------
        # Phase D pools are opened alongside C so D's input DMAs can be
        # emitted (on the otherwise-idle GPSIMD queue) right after each
        # AllToAll -- every other engine queue is still clogged with phase C
        # work at that point, and HWDGE issuance is in-order per engine.
        NKC = S // HD    # 16 key chunks
        NQT = S // 512   # 4 query tiles of 512
        NES = D // 512   # 8 output column segments
        with ExitStack() as ctx:
            qk = ctx.enter_context(tc.tile_pool(name="qk", bufs=2))
            pp = ctx.enter_context(tc.tile_pool(name="pp", bufs=6))
            ao = ctx.enter_context(tc.tile_pool(name="ao", bufs=4))
            wvf = ctx.enter_context(tc.tile_pool(name="wvf", bufs=1))
            wop = ctx.enter_context(tc.tile_pool(name="wop", bufs=4))
            oo = ctx.enter_context(tc.tile_pool(name="oo", bufs=4))

            bo_sb = wvf.tile([HD, D], F32)
            nc.sync.dma_start(bo_sb[:], bo[:])
            woT_r = woT.rearrange("(o p) e -> p o e", p=HD)  # [128, 32, D]
            wvfS = {}
            wo_first = {}

            with ExitStack() as cps:
                psl = cps.enter_context(tc.tile_pool(name="psl", bufs=4, space="PSUM"))
                pso = cps.enter_context(tc.tile_pool(name="pso", bufs=2, space="PSUM"))
                pss = cps.enter_context(tc.tile_pool(name="pss", bufs=2, space="PSUM"))

                for b in range(B):
                    ssl = slice(b * S, (b + 1) * S)
                    for hl in range(HPC):
                        esl = slice(hl * HD, (hl + 1) * HD)
                        qh = qk.tile([HD, S], MMD, tag="qh", name=f"qh_{b}_{hl}")
                        nc.sync.dma_start(qh[:], qT_d[esl, ssl])
                        kh = qk.tile([HD, S], MMD, tag="kh", name=f"kh_{b}_{hl}")
                        nc.sync.dma_start(kh[:], kT_d[esl, ssl])
                        vh = qk.tile([HD, NKC, HD], MMD, tag="vh", name=f"vh_{b}_{hl}")
                        nc.sync.dma_start(
                            vh[:], v_d[ssl, esl].rearrange("(o p) e -> p o e", p=HD))

                        for qt in range(NQT):
                            qsl = slice(qt * 512, (qt + 1) * 512)
                            po = pso.tile([HD, 512], F32, tag="po",
                                          name=f"po_{b}_{hl}_{qt}")
                            su = pss.tile([HD, 512], F32, tag="su",
                                          name=f"su_{b}_{hl}_{qt}")
                            # software-pipelined: QK for chunk kc is emitted
                            # before exp/ones/PV of chunk kc-1, so the ACT exp
                            # is never at the head of the dependency chain
                            pls = {}

                            def consume(k):
                                pe = pp.tile([HD, 512], MMD, tag="pe",
                                             name=f"pe_{b}_{hl}_{qt}_{k}")
                                nc.scalar.activation(pe[:], pls.pop(k)[:], AF.Exp,
                                                     scale=SCALE, bias=expb_sb[:])
                                nc.tensor.matmul(su[:], ones_sb[:], pe[:],
                                                 start=(k == 0),
                                                 stop=(k == NKC - 1))
                                nc.tensor.matmul(po[:], vh[:, k], pe[:],
                                                 start=(k == 0),
                                                 stop=(k == NKC - 1))

                            for kc in range(NKC):
                                pl = psl.tile([HD, 512], F32, tag="pl",
                                              name=f"pl_{b}_{hl}_{qt}_{kc}")
                                nc.tensor.matmul(
                                    pl[:], kh[:, kc * HD:(kc + 1) * HD], qh[:, qsl],
                                    start=True, stop=True)
                                pls[kc] = pl
                                if kc >= 1:
                                    consume(kc - 1)
                            consume(NKC - 1)
                            rec = ao.tile([HD, 512], F32, tag="rec",
                                          name=f"rec_{b}_{hl}_{qt}")
                            nc.vector.reciprocal_approx_fast(rec[:], su[:])
                            osb = ao.tile([HD, 512], MMD, tag="osb",
                                          name=f"osb_{b}_{hl}_{qt}")
                            nc.vector.tensor_tensor(
                                osb[:], po[:], rec[:], mybir.AluOpType.mult)
                            for j2 in range(2):
                                j = qt * 2 + j2
                                nc.sync.dma_start(
                                    cc_in[b][j, esl, :],
                                    osb[:, j2 * TPC:(j2 + 1) * TPC])
                    # batch b attention done: exchange while b+1 computes
                    nc.gpsimd.collective_compute(
                        "AllToAll", mybir.AluOpType.bypass,
                        replica_groups=[list(range(NC))],
                        ins=[cc_in[b][:]], outs=[cc_out[b][:]],
                    )
                    # prefetch phase-D inputs for this batch on the Pool queue
                    t_ = wvf.tile([HD, NDC, TPC], MMD, name=f"wvfS_{b}")
                    for i in range(NC):
                        nc.gpsimd.dma_start(
                            t_[:, i * HPC:(i + 1) * HPC, :],
                            cc_out[b][i].rearrange("(r1 p) c -> p r1 c", p=HD))
                    wvfS[b] = t_
                    wlo = wop.tile([HD, NDC // 2, 512], MMD, tag="wo",
                                   name=f"wo_pre_lo_{b}")
                    nc.gpsimd.dma_start(wlo[:], woT_r[:, 0:NDC // 2, 0:512])
                    whi = wop.tile([HD, NDC // 2, 512], MMD, tag="wo",
                                   name=f"wo_pre_hi_{b}")
                    nc.gpsimd.dma_start(whi[:], woT_r[:, NDC // 2:NDC, 0:512])
                    wo_first[b] = (wlo, whi)

            if DEBUG:
                nc.sync.dma_start(qdump[:], qT_d[:])
                nc.sync.dma_start(kdump[:], kT_d[:])
                nc.sync.dma_start(vdump[:], v_d[:])
                for b in range(B):
                    nc.sync.dma_start(ccdump[b], cc_out[b][:])

            # ------- Phase D: output projection per batch ------------------
            with ExitStack() as dps:
                ps = dps.enter_context(tc.tile_pool(name="psD", bufs=4, space="PSUM"))
                for b in range(B):
                    for es in range(NES):
                        esl = slice(es * 512, (es + 1) * 512)
                        if es == 0:
                            wo_lo, wo_hi = wo_first[b]
                        else:
                            wo_lo = wop.tile([HD, NDC // 2, 512], MMD, tag="wo")
                            nc.sync.dma_start(wo_lo[:], woT_r[:, 0:NDC // 2, esl])
                            wo_hi = wop.tile([HD, NDC // 2, 512], MMD, tag="wo")
                            nc.sync.dma_start(wo_hi[:], woT_r[:, NDC // 2:NDC, esl])
                        for tb2 in range(TPC // HD):
                            pd = ps.tile([HD, 512], F32, tag="pd",
                                         name=f"pd_{b}_{es}_{tb2}")
                            for d in range(NDC):
                                wo_t = wo_lo if d < NDC // 2 else wo_hi
                                nc.tensor.matmul(
                                    pd[:],
                                    wvfS[b][:, d, tb2 * HD:(tb2 + 1) * HD],
                                    wo_t[:, d % (NDC // 2)],
                                    start=(d == 0), stop=(d == NDC - 1))
                            ob = oo.tile([HD, 512], F32, tag="obD",
                                         name=f"obD_{b}_{es}_{tb2}")
                            nc.vector.tensor_tensor(
                                ob[:], pd[:], bo_sb[:, esl], mybir.AluOpType.add)
                            nc.sync.dma_start(
                                out[b, tb2 * HD:(tb2 + 1) * HD, esl], ob[:])

    nc.compile()
    return nc


def host_prep(x, position_ids, qkv_weight, qkv_bias, attn_out_weight,
              attn_out_bias):
    pos = np.asarray(position_ids).astype(np.int64)
    x = np.asarray(x, dtype=np.float32)
    Wqkv = np.asarray(qkv_weight, dtype=np.float32)
    bqkv = np.asarray(qkv_bias, dtype=np.float32)
    Wo = np.asarray(attn_out_weight, dtype=np.float32)
    bo = np.asarray(attn_out_bias, dtype=np.float32)

    xT = _np_mmd(x.transpose(2, 1, 0).reshape(D, T))
    woT = _np_mmd(Wo.T)

    cos_t, sin_t = _rope_tables()
    cosN = np.empty((T, HD), np.float32)
    sinN = np.empty((T, HD), np.float32)
    for b in range(B):
        rows = slice(b * S, (b + 1) * S)
        p1 = pos[b, 0, :]
        p2 = pos[b, 1, :]
        cosN[rows, 0:64] = cos_t[p1]
        cosN[rows, 64:128] = cos_t[p2]
        s1 = sin_t[p1].copy()
        s1[:, 0:32] *= -1.0
        s2 = sin_t[p2].copy()
        s2[:, 0:32] *= -1.0
        sinN[rows, 0:64] = s1
        sinN[rows, 64:128] = s2

    ones = _np_mmd(np.ones((HD, HD), np.float32))
    shared = dict(xT=xT, woT=woT, cosN=cosN, sinN=sinN, onesc=ones,
                  bo=np.ascontiguousarray(np.broadcast_to(bo, (HD, D))))

    in_maps = []
    for c in range(NC):
        heads = range(HPC * c, HPC * (c + 1))
        wq = np.concatenate([Wqkv[384 * h: 384 * h + 128] for h in heads])
        wk = np.concatenate([Wqkv[384 * h + 128: 384 * h + 256] for h in heads])
        wv = np.concatenate([Wqkv[384 * h + 256: 384 * h + 384] for h in heads])
        in_maps.append(dict(
            shared,
            wqT=_np_mmd(wq.T), wkT=_np_mmd(wk.T), wvT=_np_mmd(wv.T),
            bq=np.ascontiguousarray(np.broadcast_to(np.concatenate(
                [bqkv[384 * h: 384 * h + 128] for h in heads]), (HD, EH))),
            bk=np.ascontiguousarray(np.broadcast_to(np.concatenate(
                [bqkv[384 * h + 128: 384 * h + 256] for h in heads]), (HD, EH))),
            bv=np.ascontiguousarray(np.broadcast_to(np.concatenate(
                [bqkv[384 * h + 256: 384 * h + 384] for h in heads]), (HD, EH))),
        ))
    return in_maps


def kernel(x, position_ids, qkv_weight, qkv_bias, attn_out_weight,
           attn_out_bias, _trace=False):
    if "nc" not in _cache:
        _cache["nc"] = build_program()
    nc = _cache["nc"]

    in_maps = host_prep(x, position_ids, qkv_weight, qkv_bias,
                        attn_out_weight, attn_out_bias)
    res = run_bass_kernel_spmd(nc, in_maps, core_ids=list(range(NC)),
                               trace=_trace)
    _cache["last_result"] = res

    out = np.empty((S, B, D), np.float32)
    for c in range(NC):
        oc = res.results[c]["out"]  # [B, TPC, D]
        for b in range(B):
            out[TPC * c: TPC * (c + 1), b, :] = oc[b]
    return out
